# revision 1
# baseline (speedup 1.0000x reference)
"""2-layer GCN (PyG GCNConv semantics) on 8 Trainium2 NeuronCores.

Strategy (vertex-cut, per sharding hint):
 - nodes split contiguously across 8 cores (12500 each); edges partitioned by dst
 - symmetric norm folded into per-node scales: out[d] = dis[d]*sum_{s in N(d)} g[s] + b
   with g = (h @ W) * dis, so no per-edge weights are needed
 - per-core transform computes its node slice's g rows -> AllGather replicates the
   full fp16 gather table (rows padded to 128 cols = 256B for dma_gather)
 - aggregation: host-built slot streams (4 int16 chunks of the table-row space),
   bulk dma_gather pulls 128-slot blocks; each block is reduced onto its 64-dst
   window via a PE matmul with a host-built selection matrix S (S[slot,dst]=dis[dst])
 - per-window PSUM partials are accumulated into an SBUF accumulator, then
   bias+ReLU -> layer-2 transform -> AllGather -> same aggregation -> output
"""
import math

import numpy as np

P = 128
D = 64
NCORES = 8
CW = 32768          # int16-addressable chunk window (table rows)
NCHUNK = 4
B_SLOTS = 1024     # slots per dma_gather call (Q7 scratch limit: >1024 crashes)
WIN = 64            # dsts per S-matmul window


def _host_prep(x, edge_index, W1, b1, W2, b2, SL):
    """Build all per-core device inputs. SL = nodes per core."""
    N = x.shape[0]
    assert N == NCORES * SL
    SLP = ((SL + P - 1) // P) * P          # padded slice rows
    NT = SLP // P                           # 128-dst tiles per core
    NW = SLP // WIN                         # 64-dst windows per core
    V = NCORES * SLP                        # table rows
    n_chunks = (V + CW - 1) // CW
    assert n_chunks <= NCHUNK

    src = np.asarray(edge_index[0], dtype=np.int64)
    dst = np.asarray(edge_index[1], dtype=np.int64)
    E = src.shape[0]

    deg = np.bincount(dst, minlength=N).astype(np.float64) + 1.0
    dis = (1.0 / np.sqrt(deg)).astype(np.float32)

    def tbl(u):
        return (u // SL) * SLP + (u % SL)

    src_t_all = tbl(src)
    core_of = dst // SL

    # per-core edge lists incl self-loops
    per_core = []
    for c in range(NCORES):
        m = core_of == c
        ld = (dst[m] - c * SL).astype(np.int64)
        st = src_t_all[m]
        ld = np.concatenate([ld, np.arange(SL, dtype=np.int64)])
        st = np.concatenate([st, tbl(np.arange(SL, dtype=np.int64) + c * SL)])
        dval = dis[ld + c * SL]
        w = ld // WIN
        ch = st // CW
        order = np.lexsort((st, w, ch))
        per_core.append((ld[order], st[order], dval[order], w[order], ch[order]))

    # per-(window, chunk) block counts, maxed across cores for SPMD uniformity
    blk = np.zeros((NW, n_chunks), dtype=np.int64)
    counts_all = []
    for c in range(NCORES):
        ld, st, dval, w, ch = per_core[c]
        cnt = np.zeros((NW, n_chunks), dtype=np.int64)
        np.add.at(cnt, (w, ch), 1)
        counts_all.append(cnt)
        blk = np.maximum(blk, (cnt + P - 1) // P)
    # ensure at least.. zero blocks for empty (w,c) pairs are fine
    slots_wc = blk * P
    L_chunk = slots_wc.sum(axis=0)          # slots per chunk stream
    L_chunk_pad = ((L_chunk + P - 1) // P) * P
    tot_blocks = int(blk.sum())

    # vectorized stream/S construction
    # group key g = ch*NW + w; slot base per (w,ch) in chunk-major streams
    blkT = blk.T                                        # [n_chunks, NW]
    base_in_chunk = np.zeros((n_chunks, NW), dtype=np.int64)
    for k in range(n_chunks):
        base_in_chunk[k, :] = np.concatenate([[0], np.cumsum(blkT[k, :-1] * P)])
    blk_base_global = np.concatenate([[0], np.cumsum((blkT * 1).reshape(-1))])
    # global block index base for (k, w):
    gblk_base = np.cumsum(np.concatenate([[0], blkT.reshape(-1)[:-1]])).reshape(
        n_chunks, NW
    )

    S_dev = np.zeros((NCORES, P, tot_blocks * WIN), dtype=np.float16)
    idx_wrapped = [
        [np.zeros((P, L_chunk_pad[k] // 16), dtype=np.int16) for k in range(n_chunks)]
        for _ in range(NCORES)
    ]
    for c in range(NCORES):
        ld, st, dval, w, ch = per_core[c]
        g = ch * NW + w
        # rank within group
        starts = np.searchsorted(g, np.arange(n_chunks * NW))
        r = np.arange(g.shape[0]) - starts[g]
        slot = base_in_chunk[ch, w] + r                 # position in chunk stream
        gb = gblk_base[ch, w] + r // P                  # global block id
        sip = r % P                                     # slot in block (partition)
        scol = ld - w * WIN
        S_dev[c, sip, gb * WIN + scol] = dval.astype(np.float16)
        for k in range(n_chunks):
            m = ch == k
            s = np.zeros(L_chunk_pad[k], dtype=np.int16)
            s[slot[m]] = (st[m] - k * CW).astype(np.int16)
            wrp = s.reshape(-1, 16).T                   # [16, L/16]
            idx_wrapped[c][k] = np.tile(wrp, (8, 1)).astype(np.int16)

    # schedule metadata for codegen: per chunk, ordered (window, nblocks)
    sched = []
    for k in range(n_chunks):
        rows = [(wi, int(blk[wi, k])) for wi in range(NW) if blk[wi, k] > 0]
        sched.append(rows)

    # transform inputs
    xT = np.zeros((NCORES, D, SLP), dtype=np.float32)
    dis_sb = np.zeros((NCORES, P, NT), dtype=np.float32)
    for c in range(NCORES):
        xs = np.asarray(x[c * SL : (c + 1) * SL], dtype=np.float32)
        xT[c, :, :SL] = xs.T
        dp = np.zeros(SLP, dtype=np.float32)
        dp[:SL] = dis[c * SL : (c + 1) * SL]
        dis_sb[c] = dp.reshape(NT, P).T

    b1b = np.tile(np.asarray(b1, np.float32)[None, :], (P, 1))
    b2b = np.tile(np.asarray(b2, np.float32)[None, :], (P, 1))

    meta = dict(
        SL=SL, SLP=SLP, NT=NT, NW=NW, V=V, n_chunks=n_chunks,
        L_chunk=[int(v) for v in L_chunk_pad], tot_blocks=tot_blocks, sched=sched,
    )
    inputs = dict(
        xT=xT, dis_sb=dis_sb, S=S_dev, idx=idx_wrapped, b1b=b1b, b2b=b2b,
        W1=np.asarray(W1, np.float32), W2=np.asarray(W2, np.float32),
    )
    return meta, inputs


def _build_kernel(meta, timing_trips=0):
    import concourse.bass as bass
    import concourse.bacc as bacc
    import concourse.mybir as mybir
    import concourse.tile as tile
    from concourse.masks import make_identity

    SLP, NT, NW, V = meta["SLP"], meta["NT"], meta["NW"], meta["V"]
    n_chunks, L_chunk, sched = meta["n_chunks"], meta["L_chunk"], meta["sched"]
    tot_blocks = meta["tot_blocks"]
    f32, f16, i16, i32 = (mybir.dt.float32, mybir.dt.float16, mybir.dt.int16,
                          mybir.dt.int32)

    nc = bacc.Bacc("TRN2", target_bir_lowering=False, debug=False,
                   num_devices=NCORES)

    xT_t = nc.dram_tensor("xT", [D, SLP], f32, kind="ExternalInput")
    dis_t = nc.dram_tensor("dis_sb", [P, NT], f32, kind="ExternalInput")
    S_t = nc.dram_tensor("S", [P, tot_blocks * WIN], f16, kind="ExternalInput")
    idx_ts = [
        nc.dram_tensor(f"idx{k}", [P, L_chunk[k] // 16], i16, kind="ExternalInput")
        for k in range(n_chunks)
    ]
    W1_t = nc.dram_tensor("W1", [D, D], f32, kind="ExternalInput")
    W2_t = nc.dram_tensor("W2", [D, D], f32, kind="ExternalInput")
    b1b_t = nc.dram_tensor("b1b", [P, D], f32, kind="ExternalInput")
    b2b_t = nc.dram_tensor("b2b", [P, D], f32, kind="ExternalInput")
    out_t = nc.dram_tensor("out", [SLP, D], f32, kind="ExternalOutput")

    with tile.TileContext(nc) as tc:
        with (
            tc.tile_pool(name="const", bufs=1) as cp,
            tc.tile_pool(name="io", bufs=3) as iop,
            tc.tile_pool(name="gat", bufs=3) as gp,
            tc.tile_pool(name="spool", bufs=3) as sp,
            tc.tile_pool(name="acc", bufs=1) as ap,
            tc.tile_pool(name="psum", bufs=4, space="PSUM") as pp,
            tc.tile_pool(name="tps", bufs=2, space="PSUM") as tpp,
            tc.tile_pool(name="dram", bufs=1, space="DRAM") as dp,
        ):
            # ---- constants ----
            W1_sb = cp.tile([D, D], f32)
            W2_sb = cp.tile([D, D], f32)
            b1_sb = cp.tile([P, D], f32)
            b2_sb = cp.tile([P, D], f32)
            dis_sb = cp.tile([P, NT], f32)
            xT_sb = cp.tile([D, SLP], f32)
            ident = cp.tile([P, P], f32)
            nc.sync.dma_start(out=W1_sb[:], in_=W1_t[:])
            nc.sync.dma_start(out=W2_sb[:], in_=W2_t[:])
            nc.sync.dma_start(out=b1_sb[:], in_=b1b_t[:])
            nc.sync.dma_start(out=b2_sb[:], in_=b2b_t[:])
            nc.sync.dma_start(out=dis_sb[:], in_=dis_t[:])
            nc.sync.dma_start(out=xT_sb[:], in_=xT_t[:])
            make_identity(nc, ident[:])

            # DRAM bounce buffers (collectives need internal tiles)
            g_slice = dp.tile([SLP, P], f16)
            g1_full = dp.tile([V, P], f16)
            g2_full = dp.tile([V, P], f16)

            # accumulators
            h1pre = ap.tile([P, NT * D], f32)
            h2pre = ap.tile([P, NT * D], f32)

            # dummy indirect dma so walrus configures the pool-dynamic ring
            # (required for dma_gather to run)
            idx32_sb = cp.tile([P, 1], i32)
            dummy_sb = cp.tile([P, D], f32)
            nc.vector.memset(idx32_sb[:], 0)
            nc.gpsimd.indirect_dma_start(
                out=dummy_sb[:], out_offset=None, in_=b1b_t[:],
                in_offset=bass.IndirectOffsetOnAxis(ap=idx32_sb[:], axis=0),
            )

            def transform(src_kind, j):
                """produce g tile [128,128] f16 (cols 64: zero) for node tile j"""
                if src_kind == 1:
                    lhsT = xT_sb[:, j * P : (j + 1) * P]
                    W_sb = W1_sb
                else:
                    # h1 tile -> bias+relu -> transpose -> matmul W2
                    t0 = iop.tile([P, D], f32, tag="t0")
                    nc.vector.tensor_tensor(
                        out=t0[:], in0=h1pre[:, j * D : (j + 1) * D],
                        in1=b1_sb[:], op=mybir.AluOpType.add,
                    )
                    h1 = iop.tile([P, D], f32, tag="h1")
                    nc.scalar.activation(
                        out=h1[:], in_=t0[:],
                        func=mybir.ActivationFunctionType.Relu,
                    )
                    tps = tpp.tile([D, P], f32, tag="tps")
                    nc.tensor.transpose(out=tps[:], in_=h1[:], identity=ident[:])
                    h1T = iop.tile([D, P], f32, tag="h1T")
                    nc.vector.tensor_copy(out=h1T[:], in_=tps[:])
                    lhsT = h1T[:]
                    W_sb = W2_sb
                ps = tpp.tile([P, D], f32, tag="tmm")
                nc.tensor.matmul(out=ps[:], lhsT=lhsT, rhs=W_sb[:],
                                 start=True, stop=True)
                gt = iop.tile([P, P], f16, tag="gt")
                nc.vector.memset(gt[:, D:], 0)
                nc.vector.tensor_scalar(
                    out=gt[:, :D], in0=ps[:], scalar1=dis_sb[:, j : j + 1],
                    scalar2=None, op0=mybir.AluOpType.mult,
                )
                nc.sync.dma_start(out=g_slice[j * P : (j + 1) * P, :], in_=gt[:])

            def allgather(dst_full):
                if NCORES == 1:
                    nc.sync.dma_start(out=dst_full[:], in_=g_slice[:])
                else:
                    nc.gpsimd.collective_compute(
                        "AllGather", mybir.AluOpType.bypass,
                        replica_groups=[list(range(NCORES))],
                        ins=[g_slice.opt()], outs=[dst_full.opt()],
                    )

            def aggregate(table_full, acc):
                nc.vector.memset(acc[:], 0)
                bi = 0  # global block index (S layout)
                for k in range(n_chunks):
                    L = L_chunk[k]
                    ncalls = (L + B_SLOTS - 1) // B_SLOTS
                    # per-call gathered tiles
                    call_tiles = []
                    for j in range(ncalls):
                        o = j * B_SLOTS
                        n = min(B_SLOTS, L - o)
                        it = gp.tile([P, B_SLOTS // 16], i16, tag="idx")
                        nc.sync.dma_start(
                            out=it[:, : n // 16],
                            in_=idx_ts[k][:, o // 16 : (o + n) // 16],
                        )
                        gt = gp.tile([P, B_SLOTS // P, P], f16, tag="g")
                        nc.gpsimd.dma_gather(
                            gt[:, : n // P, :],
                            table_full[k * CW : min((k + 1) * CW, V), :],
                            it[:, : n // 16], n, n, P,
                        )
                        call_tiles.append(gt)
                    # S tiles + matmuls per window group
                    for wi, nb in sched[k]:
                        St = sp.tile([P, nb * WIN], f16, tag="S")
                        nc.sync.dma_start(
                            out=St[:, : nb * WIN],
                            in_=S_t[:, bi * WIN : (bi + nb) * WIN],
                        )
                        ps = pp.tile([P, D], f32, tag="ps")
                        half = (wi % 2) * WIN
                        out_ps = ps[half : half + WIN, :]
                        for b in range(nb):
                            gslot = bi - _chunk_block_base[k] + b
                            ct = call_tiles[gslot // (B_SLOTS // P)]
                            s_in_call = gslot % (B_SLOTS // P)
                            nc.tensor.matmul(
                                out=out_ps,
                                lhsT=St[:, b * WIN : (b + 1) * WIN],
                                rhs=ct[:, s_in_call, :D],
                                start=(b == 0), stop=(b == nb - 1),
                            )
                        wcol = (wi // 2) * D
                        nc.vector.tensor_tensor(
                            out=acc[half : half + WIN, wcol : wcol + D],
                            in0=acc[half : half + WIN, wcol : wcol + D],
                            in1=out_ps, op=mybir.AluOpType.add,
                        )
                        bi += nb

            # block-index base per chunk (for call slot math)
            _chunk_block_base = []
            acc_b = 0
            for k in range(n_chunks):
                _chunk_block_base.append(acc_b)
                acc_b += sum(nb for _, nb in sched[k])

            # ---- pipeline ----
            def tail():
                for j in range(NT):
                    transform(2, j)

            def outs():
                for j in range(NT):
                    ot = iop.tile([P, D], f32, tag="ot")
                    nc.vector.tensor_tensor(
                        out=ot[:], in0=h2pre[:, j * D : (j + 1) * D], in1=b2_sb[:],
                        op=mybir.AluOpType.add,
                    )
                    nc.sync.dma_start(out=out_t[j * P : (j + 1) * P, :], in_=ot[:])

            if timing_trips:
                # timing-only build: collectives hoisted; loop over compute body
                for j in range(NT):
                    transform(1, j)
                allgather(g1_full)
                allgather(g2_full)   # g2 table = copy of g1 rows (valid fp16)
                with tc.For_i(0, timing_trips, 1):
                    aggregate(g1_full, h1pre)
                    tail()
                    aggregate(g2_full, h2pre)
                    outs()
            else:
                for j in range(NT):
                    transform(1, j)
                allgather(g1_full)
                aggregate(g1_full, h1pre)
                tail()
                allgather(g2_full)
                aggregate(g2_full, h2pre)
                outs()

    nc.compile()
    return nc


def kernel(x, edge_index, W1, b1, W2, b2):
    import concourse.bass_utils as bass_utils

    x = np.asarray(x)
    N = x.shape[0]
    SL = N // NCORES
    meta, inp = _host_prep(x, edge_index, W1, b1, W2, b2, SL)
    nc = _build_kernel(meta)

    in_maps = []
    for c in range(NCORES):
        m = {
            "xT": inp["xT"][c], "dis_sb": inp["dis_sb"][c], "S": inp["S"][c],
            "W1": inp["W1"], "W2": inp["W2"], "b1b": inp["b1b"], "b2b": inp["b2b"],
        }
        for k in range(meta["n_chunks"]):
            m[f"idx{k}"] = inp["idx"][c][k]
        in_maps.append(m)

    res = bass_utils.run_bass_kernel_spmd(nc, in_maps, core_ids=list(range(NCORES)))
    out = np.empty((N, D), dtype=np.float32)
    for c in range(NCORES):
        out[c * SL : (c + 1) * SL] = res.results[c]["out"][:SL]
    return out

